# revision 1
# baseline (speedup 1.0000x reference)
"""Trainium2 Bass kernel for nn_ConcatAttention_Param.

Reference computation (per batch b):
    pre[l,h] = sum_i h[b,l,i] * W_h[h,i] + bias[h]     (W_h = ln_w[:, :I], bias = ln_b + W_vq @ vq)
    s[l]     = tanh(pre[l,:]) @ v_w
    s       += -10000 * (~mask[b,l])
    a        = softmax(s over l)
    r[b,:]   = sum_l a[l] * h[b,l,:]

Strategy: data-parallel over batch (4 batches per core x 8 cores). On each
core, for each batch:
  pass 1: stream h in transposed layout (i on partitions), compute
          pre.T = W_hT.T @ hT on PE in fp32r (full-rate), tanh+bias on ACT,
          dot with v_w via M=1 matmuls on PE -> s.
  softmax: mask-add, max, exp (with accumulated sum) on DVE/ACT, all on a
          single partition (tiny). exp(s-max) round-trips through DRAM to
          transpose into l-on-partition columns.
  pass 2: stream h in natural layout (l on partitions), accumulate
          r_hat = sum_l e_l h[l,:] via M=1 matmuls with e as stationary
          operand; scale by 1/sum(e) at the end.

All matmuls use fp32r (fp32 data, mantissa-rounded matmul at 1 cycle/row for
N>=256 vs 4 cycles/row for plain fp32).
"""

import os
from contextlib import ExitStack

import numpy as np

import jax


import concourse.bass as bass
import concourse.tile as tile
from concourse import bacc, mybir

# Problem constants (hardcoded per contract; kernel.py may not read spec.json)
B_FULL = 32
L = 2048
I = 1024
H = 1024
N_CORES = 8
B_PC = B_FULL // N_CORES  # batches per core

LG = 512            # l-group (moving-operand columns per matmul)
P = 128             # partitions
IC = I // P         # i chunks
HC = H // P         # h' chunks
FR = mybir.dt.float32r
F32 = mybir.dt.float32

# NOTE: col-tiling (tile_position) is rejected by walrus for fp32r matmuls
# ('s3d3_mm_valid_dst_partition'), and memset cannot write fp32r tiles.
# Packing would need bf16 operands (accuracy cost) — keep off.
PACK_S = os.environ.get("K_PACK_S", "0") == "1"   # col-packed score dots
PACK_P2 = os.environ.get("K_PACK_P2", "0") == "1"  # col-packed pass-2
AMORT = os.environ.get("K_AMORT", "0") == "1"      # amortize W loads over 2 l-groups


def build_module(b_pc: int = B_PC, seq: int = L, stage: str = "full"):
    """Build the per-core Bass module (same program on every core).

    stage: "full" | "pass1" (dump scores s) | "softmax" (dump e_col + 1/d)
    — truncated variants for hardware bisection.
    """
    n_lg = seq // LG
    n_lt = seq // P   # l-tiles of 128 for pass 2

    nc = bacc.Bacc("TRN2", target_bir_lowering=False, debug=False,
                   enable_asserts=False, num_devices=N_CORES)

    hT_d = nc.dram_tensor("hT", (b_pc, I, seq), FR, kind="ExternalInput").ap()
    hN_d = nc.dram_tensor("hN", (b_pc, seq, I), FR, kind="ExternalInput").ap()
    w_d = nc.dram_tensor("w_hT", (I, H), FR, kind="ExternalInput").ap()
    vw_d = nc.dram_tensor("v_w", (H,), FR, kind="ExternalInput").ap()
    bias_d = nc.dram_tensor("bias", (H,), F32, kind="ExternalInput").ap()
    madd_d = nc.dram_tensor("m_add", (b_pc, seq), F32, kind="ExternalInput").ap()
    reps_d = nc.dram_tensor("reps", (1, 1), mybir.dt.int32,
                            kind="ExternalInput").ap()
    r_d = nc.dram_tensor("r", (b_pc, I), F32, kind="ExternalOutput").ap()
    s_dump = e_dump = d_dump = None
    if stage == "pass1":
        s_dump = nc.dram_tensor("s_dump", (b_pc, seq), F32,
                                kind="ExternalOutput").ap()
    elif stage == "softmax":
        e_dump = nc.dram_tensor("e_dump", (b_pc, P, seq // P), F32,
                                kind="ExternalOutput").ap()
        d_dump = nc.dram_tensor("d_dump", (b_pc, 1), F32,
                                kind="ExternalOutput").ap()

    with tile.TileContext(nc) as tc, ExitStack() as ctx:
        const_p = ctx.enter_context(tc.tile_pool(name="const", bufs=1))
        hT_p = ctx.enter_context(tc.tile_pool(name="hT", bufs=32 if AMORT else 16))
        tanh_p = ctx.enter_context(tc.tile_pool(name="tanh", bufs=18 if AMORT else 20))
        hN_p = ctx.enter_context(tc.tile_pool(name="hN", bufs=6))
        small_p = ctx.enter_context(tc.tile_pool(name="small", bufs=2))
        pre_ps = ctx.enter_context(tc.tile_pool(name="preps", bufs=4, space="PSUM"))
        s_ps = ctx.enter_context(tc.tile_pool(name="sps", bufs=2, space="PSUM"))
        r_ps = ctx.enter_context(tc.tile_pool(name="rps", bufs=1, space="PSUM"))
        dram_p = ctx.enter_context(tc.tile_pool(name="edram", bufs=2, space="DRAM"))

        # --- resident constants ---
        ones97 = None
        if PACK_S:
            # combine weights: 1.0 at partitions {0,32,64,96}, 0 elsewhere
            # (NB: memset cannot write fp32r per walrus ISA check)
            ones97 = const_p.tile([97, 1], FR, tag="ones97")
            nc.vector.memset(ones97[:], 0.0)
            for j in range(4):
                nc.vector.memset(ones97[32 * j:32 * j + 1, :], 1.0)
        w_sb = const_p.tile([P, IC * H], FR, tag="W")  # [p, ic*H + h]
        nc.sync.dma_start(
            out=w_sb[:].rearrange("p (ic h) -> p ic h", ic=IC),
            in_=w_d.rearrange("(ic p) h -> p ic h", p=P),
        )
        vw_sb = const_p.tile([P, HC], FR, tag="vw")    # [q, hc]
        nc.sync.dma_start(out=vw_sb[:], in_=vw_d.rearrange("(hc q) -> q hc", q=P))
        bias_sb = const_p.tile([P, HC], F32, tag="bias")
        nc.sync.dma_start(out=bias_sb[:], in_=bias_d.rearrange("(hc q) -> q hc", q=P))

        def pass1(b):
            """Compute masked scores s for batch b into an sbuf tile (1, seq)."""
            m_sb = small_p.tile([1, seq], F32, tag="madd")
            nc.sync.dma_start(out=m_sb[:], in_=madd_d[b:b + 1, :])
            s_sb = small_p.tile([1, seq], F32, tag="s")

            def drain_s(g, sg):
                # scores of group g to sbuf, adding the mask bias
                nc.vector.tensor_add(
                    s_sb[0:1, g * LG:(g + 1) * LG], sg[:],
                    m_sb[0:1, g * LG:(g + 1) * LG])

            def emit_spack(g, tanh_tiles):
                # 4-wide col-packed dot: chunks hc and hc+4 accumulate at
                # psum partition 32*(hc%4); the 4 col-groups run concurrently
                sgp = s_ps.tile([P, LG], F32, tag="spack")
                # rows other than {0,32,64,96} are read (x0.0) by the combine
                # matmul; clear them so every read byte is written by this tile
                nc.vector.memset(sgp[:], 0.0)
                for hc in range(HC):
                    j = hc % 4
                    nc.tensor.matmul(
                        sgp[32 * j:32 * j + 1, :], vw_sb[:, hc:hc + 1],
                        tanh_tiles[hc][:],
                        start=(hc < 4), stop=(hc >= 4),
                        tile_position=(0, 32 * j))
                return sgp

            def emit_combine(g, sgp):
                part = small_p.tile([97, LG], FR, tag="spart")
                nc.vector.tensor_copy(part[:], sgp[0:97, :])
                sg = s_ps.tile([1, LG], F32, tag="sps", bufs=1)
                nc.tensor.matmul(sg[:], ones97[:], part[:],
                                 start=True, stop=True)
                drain_s(g, sg)

            pend_s = []      # [(g, tanh_tiles)] awaiting packed-s emission
            pend_comb = []   # [(g, sgp)] awaiting combine emission
            for g in range(n_lg):
                hT_tiles = []
                for ic in range(IC):
                    t = hT_p.tile([P, LG], FR, tag="hT")
                    nc.sync.dma_start(
                        out=t[:], in_=hT_d[b, ic * P:(ic + 1) * P, g * LG:(g + 1) * LG])
                    hT_tiles.append(t)
                tanh_tiles = []
                for hc in range(HC):
                    pre = pre_ps.tile([P, LG], F32, tag="pre")
                    for ic in range(IC):
                        nc.tensor.matmul(
                            pre[:],
                            w_sb[:, ic * H + hc * P: ic * H + (hc + 1) * P],
                            hT_tiles[ic][:],
                            start=(ic == 0), stop=(ic == IC - 1),
                        )
                    th = tanh_p.tile([P, LG], FR, tag="tanh")
                    nc.scalar.activation(
                        th[:], pre[:], mybir.ActivationFunctionType.Tanh,
                        bias=bias_sb[:, hc:hc + 1], scale=1.0)
                    tanh_tiles.append(th)
                if PACK_S:
                    # one-group-deferred emission hides the DVE copy latency
                    # between the packed mms and the combine matmul
                    if pend_comb:
                        emit_combine(*pend_comb.pop())
                    if pend_s:
                        gg, tt = pend_s.pop()
                        pend_comb.append((gg, emit_spack(gg, tt)))
                    pend_s.append((g, tanh_tiles))
                else:
                    sg = s_ps.tile([1, LG], F32, tag="sps")
                    for hc in range(HC):
                        nc.tensor.matmul(
                            sg[:], vw_sb[:, hc:hc + 1], tanh_tiles[hc][:],
                            start=(hc == 0), stop=(hc == HC - 1))
                    drain_s(g, sg)
            if PACK_S:
                while pend_s or pend_comb:
                    if pend_comb:
                        emit_combine(*pend_comb.pop())
                    if pend_s:
                        gg, tt = pend_s.pop()
                        pend_comb.append((gg, emit_spack(gg, tt)))
            return s_sb

        def pass1_amort(b):
            """Like pass1, but keeps each W tile stationary across 2 l-groups
            (halves LDWEIGHTS traffic on the PE)."""
            m_sb = small_p.tile([1, seq], F32, tag="madd")
            nc.sync.dma_start(out=m_sb[:], in_=madd_d[b:b + 1, :])
            s_sb = small_p.tile([1, seq], F32, tag="s")
            for gp in range(n_lg // 2):
                gg2 = (2 * gp, 2 * gp + 1)
                hts = {}
                for ic in range(IC):
                    for j, g in enumerate(gg2):
                        t = hT_p.tile([P, LG], FR, tag="hT", name="hTt")
                        nc.sync.dma_start(
                            out=t[:],
                            in_=hT_d[b, ic * P:(ic + 1) * P, g * LG:(g + 1) * LG])
                        hts[(j, ic)] = t
                tanh_tiles = {}
                for hc in range(HC):
                    pres = [pre_ps.tile([P, LG], F32, tag=f"pre{j}",
                                        name=f"pre{j}", bufs=2)
                            for j in range(2)]
                    for ic in range(IC):
                        for j in range(2):
                            nc.tensor.matmul(
                                pres[j][:],
                                w_sb[:, ic * H + hc * P: ic * H + (hc + 1) * P],
                                hts[(j, ic)][:],
                                start=(ic == 0), stop=(ic == IC - 1))
                    for j in range(2):
                        th = tanh_p.tile([P, LG], FR, tag="tanh", name="tanht")
                        nc.scalar.activation(
                            th[:], pres[j][:], mybir.ActivationFunctionType.Tanh,
                            bias=bias_sb[:, hc:hc + 1], scale=1.0)
                        tanh_tiles[(j, hc)] = th
                for j, g in enumerate(gg2):
                    sg = s_ps.tile([1, LG], F32, tag="sps", name="sg")
                    for hc in range(HC):
                        nc.tensor.matmul(
                            sg[:], vw_sb[:, hc:hc + 1], tanh_tiles[(j, hc)][:],
                            start=(hc == 0), stop=(hc == HC - 1))
                    nc.vector.tensor_add(
                        s_sb[0:1, g * LG:(g + 1) * LG], sg[:],
                        m_sb[0:1, g * LG:(g + 1) * LG])
            return s_sb

        def softmax(s_sb):
            """exp(s - max) -> e_col (l on partitions) + 1/sum(e)."""
            negm = small_p.tile([1, 1], F32, tag="negm")
            nc.vector.reduce_max(negm[:], s_sb[:], axis=mybir.AxisListType.X,
                                 negate=True)
            e_sb = small_p.tile([1, seq], FR, tag="e")
            d_sb = small_p.tile([1, 1], F32, tag="d")
            nc.scalar.activation(
                e_sb[:], s_sb[:], mybir.ActivationFunctionType.Exp,
                bias=negm[0:1, 0:1], scale=1.0, accum_out=d_sb[:])
            rd = small_p.tile([1, 1], F32, tag="rd")
            nc.vector.reciprocal(rd[:], d_sb[:])
            # transpose e (1, seq) -> (128, n_lt) via a DRAM round-trip
            e_dram = dram_p.tile([1, seq], FR, tag="edram")
            nc.sync.dma_start(out=e_dram[:], in_=e_sb[:])
            e_col = small_p.tile([P, n_lt], FR, tag="ecol")
            nc.sync.dma_start(
                out=e_col[:], in_=e_dram[:].rearrange("o (lt p) -> p (o lt)", p=P))
            return e_col, rd

        def pass2(b, e_col, rd):
            """r[b] = (1/d) * sum_l e_l h[b,l,:] via PE accumulation."""
            r_sb = small_p.tile([1, I], F32, tag="rsb")
            n_ih = I // LG
            if PACK_P2:
                # both i-halves packed into one psum bank at partitions 0/32;
                # the two col-groups run concurrently on the PE
                rpk = r_ps.tile([P, LG], F32, tag="rpack")
                rps = [rpk[32 * ih:32 * ih + 1, :] for ih in range(n_ih)]
            else:
                rps = [r_ps.tile([1, LG], F32, tag=f"rps{ih}", name=f"rps{ih}")[:]
                       for ih in range(n_ih)]
            for lt in range(n_lt):
                hn = hN_p.tile([P, I], FR, tag="hN")
                nc.sync.dma_start(out=hn[:], in_=hN_d[b, lt * P:(lt + 1) * P, :])
                for ih in range(n_ih):
                    nc.tensor.matmul(
                        rps[ih], e_col[:, lt:lt + 1], hn[:, ih * LG:(ih + 1) * LG],
                        start=(lt == 0), stop=(lt == n_lt - 1),
                        tile_position=(0, 32 * ih) if PACK_P2 else None)
            for ih in range(n_ih):
                nc.vector.tensor_scalar_mul(
                    r_sb[0:1, ih * LG:(ih + 1) * LG], rps[ih], rd[0:1, 0:1])
            nc.sync.dma_start(out=r_d[b:b + 1, :], in_=r_sb[:])

        # software pipeline: pass2(b) is emitted after pass1(b+1) so the PE
        # never waits on the softmax chain
        p1 = pass1_amort if AMORT else pass1

        def body():
            pending = None
            for b in range(b_pc):
                s_sb = p1(b)
                ecol_rd = softmax(s_sb)
                if pending is not None:
                    pass2(pending[0], *pending[1])
                pending = (b, ecol_rd)
            pass2(pending[0], *pending[1])

        def stage_body():
            if stage == "pass1":
                for b in range(b_pc):
                    s_sb = p1(b)
                    nc.sync.dma_start(out=s_dump[b:b + 1, :], in_=s_sb[:])
            elif stage == "softmax":
                for b in range(b_pc):
                    s_sb = p1(b)
                    e_col, rd = softmax(s_sb)
                    ec_f32 = small_p.tile([P, seq // P], F32, tag="ecf")
                    nc.vector.tensor_copy(ec_f32[:], e_col[:])
                    nc.sync.dma_start(out=e_dump[b], in_=ec_f32[:])
                    nc.sync.dma_start(out=d_dump[b:b + 1, :], in_=rd[:])
            else:
                body()

        reps_sb = const_p.tile([1, 1], mybir.dt.int32, tag="reps")
        nc.sync.dma_start(out=reps_sb[:], in_=reps_d)
        reps_val = nc.values_load(reps_sb[0:1, 0:1], min_val=1,
                                  max_val=1 << 20,
                                  skip_runtime_bounds_check=True)
        with tc.For_i(0, reps_val, 1):
            stage_body()

    nc.compile()
    return nc


# ---------------------------------------------------------------------------
# Host-side runtime: shard, upload, execute via PJRT (axon), gather.
# ---------------------------------------------------------------------------

class _Runtime:
    def __init__(self, nc, n_cores=N_CORES):
        from concourse import bass2jax
        from jax.sharding import Mesh, PartitionSpec, NamedSharding
        from jax.experimental.shard_map import shard_map

        bass2jax.install_neuronx_cc_hook()
        self.nc = nc
        self.n_cores = n_cores

        partition_name = (nc.partition_id_tensor.name
                          if nc.partition_id_tensor else None)
        in_names, out_names, out_avals, zero_shapes = [], [], [], []
        for alloc in nc.m.functions[0].allocations:
            if not isinstance(alloc, mybir.MemoryLocationSet):
                continue
            name = alloc.memorylocations[0].name
            if alloc.kind == "ExternalInput":
                if name != partition_name:
                    in_names.append(name)
            elif alloc.kind == "ExternalOutput":
                shape = tuple(alloc.tensor_shape)
                dtype = mybir.dt.np(alloc.dtype)
                out_names.append(name)
                out_avals.append(jax.core.ShapedArray(shape, dtype))
                zero_shapes.append((shape, dtype))
        self.in_names = list(in_names)
        self.out_names = out_names
        self.out_avals = out_avals
        self.zero_shapes = zero_shapes
        n_params = len(in_names)
        n_outs = len(out_names)
        all_names = in_names + out_names
        if partition_name is not None:
            all_names = all_names + [partition_name]

        from concourse.bass2jax import _bass_exec_p, partition_id_tensor

        def _body(*args):
            operands = list(args)
            if partition_name is not None:
                operands.append(partition_id_tensor())
            outs = _bass_exec_p.bind(
                *operands,
                out_avals=tuple(out_avals),
                in_names=tuple(all_names),
                out_names=tuple(out_names),
                lowering_input_output_aliases=(),
                sim_require_finite=False,
                sim_require_nnan=False,
                nc=nc,
            )
            return tuple(outs)

        devices = jax.devices()[:n_cores]
        self.mesh = Mesh(np.asarray(devices), ("core",))
        pspec = PartitionSpec("core")
        self.sharding = NamedSharding(self.mesh, pspec)
        donate = tuple(range(n_params, n_params + n_outs))
        self.fn = jax.jit(
            shard_map(_body, mesh=self.mesh,
                      in_specs=(pspec,) * (n_params + n_outs),
                      out_specs=(pspec,) * n_outs,
                      check_rep=False),
            donate_argnums=donate, keep_unused=True)

    def put_inputs(self, in_maps):
        concat = [
            np.concatenate([np.asarray(m[name]) for m in in_maps], axis=0)
            for name in self.in_names
        ]
        return [jax.device_put(a, self.sharding) for a in concat]

    def run(self, dev_inputs):
        zeros = [
            jax.device_put(np.zeros((self.n_cores * s[0], *s[1:]), dt), self.sharding)
            for s, dt in self.zero_shapes
        ]
        outs = self.fn(*dev_inputs, *zeros)
        jax.block_until_ready(outs)
        return outs

    def gather(self, outs):
        res = []
        for c in range(self.n_cores):
            d = {}
            for i, name in enumerate(self.out_names):
                d[name] = np.asarray(outs[i]).reshape(
                    self.n_cores, *self.out_avals[i].shape)[c]
            res.append(d)
        return res


_CACHE = {}


def _get_runtime():
    if "rt" not in _CACHE:
        nc = build_module()
        _CACHE["rt"] = _Runtime(nc)
    return _CACHE["rt"]


def prep_in_maps(h, mask, ln_w, ln_b, v_w, vq, reps=1):
    """Host-side preprocessing + sharding into per-core input maps."""
    h = np.asarray(h, dtype=np.float32)
    mask = np.asarray(mask)
    ln_w = np.asarray(ln_w, dtype=np.float32)
    ln_b = np.asarray(ln_b, dtype=np.float32)
    v_w = np.asarray(v_w, dtype=np.float32)
    vq = np.asarray(vq, dtype=np.float32)

    w_hT = np.ascontiguousarray(ln_w[:, :I].T)          # (I, H)
    bias = ln_b + ln_w[:, I:] @ vq                      # (H,)
    m_add = (mask.astype(np.float32) - 1.0) * 10000.0   # 0 / -10000

    in_maps = []
    for c in range(N_CORES):
        hb = h[c * B_PC:(c + 1) * B_PC]
        in_maps.append({
            "hT": np.ascontiguousarray(hb.transpose(0, 2, 1)),
            "hN": hb,
            "w_hT": w_hT,
            "v_w": v_w,
            "bias": bias,
            "m_add": m_add[c * B_PC:(c + 1) * B_PC],
            "reps": np.full((1, 1), reps, np.int32),
        })
    return in_maps


def kernel(h, mask, ln_w, ln_b, v_w, vq):
    rt = _get_runtime()
    in_maps = prep_in_maps(h, mask, ln_w, ln_b, v_w, vq)
    last_err = None
    for attempt in range(3):
        try:
            dev = rt.put_inputs(in_maps)
            outs = rt.run(dev)
            res = rt.gather(outs)
            return np.concatenate([res[c]["r"] for c in range(N_CORES)], axis=0)
        except Exception as e:  # transient device-unrecoverable flakes
            last_err = e
            _CACHE.pop("rt", None)
            jax.clear_caches()
            rt = _get_runtime()
    raise last_err



# revision 2
# speedup vs baseline: 1.8167x; 1.8167x over previous
"""Trainium2 Bass kernel for nn_ConcatAttention_Param.

Reference computation (per batch b):
    pre[l,h] = sum_i h[b,l,i] * W_h[h,i] + bias[h]     (W_h = ln_w[:, :I], bias = ln_b + W_vq @ vq)
    s[l]     = tanh(pre[l,:]) @ v_w
    s       += -10000 * (~mask[b,l])
    a        = softmax(s over l)
    r[b,:]   = sum_l a[l] * h[b,l,:]

Key optimizations over a dense implementation:
  * Mask compaction: masked positions get s-10000, whose exp underflows to
    exactly 0 in fp32, so they contribute nothing to the softmax or to r.
    The host gathers only the unmasked rows of h (~
    half of L), padded to a multiple of 128 (padding rows get h=0 and a
    -30000 mask-add so their exp is exactly 0 too). This halves PE work,
    DMA traffic and tanh work. If a batch is fully masked, softmax(s-1e4)
    == softmax(s), so the fallback keeps all rows with no mask-add.
  * bf16 operands on the PE (measured end-to-end rel err ~3e-3 vs 2e-2
    budget). bf16 enables fast-weight-load and tile_position col-packing,
    both rejected for fp32r.
  * Score dots (M=1) run 4-wide col-packed (concurrent col-groups via
    separate XBUSes), combined with a ones-vector matmul.
  * Pass 2 (r = e @ h) runs 4-wide col-packed at N=256.
  * Software pipeline: pass2(b) is emitted after pass1(b+1)'s matmuls so
    the PE never waits on the softmax chain.

Data-parallel over batch: 4 batches per core x 8 cores.
"""

from contextlib import ExitStack

import numpy as np
import ml_dtypes

import jax


import concourse.bass as bass
import concourse.tile as tile
from concourse import bacc, mybir

# Problem constants (hardcoded per contract; kernel.py may not read spec.json)
B_FULL = 32
L = 2048
I = 1024
H = 1024
N_CORES = 8
B_PC = B_FULL // N_CORES  # batches per core

LG = 512            # max l-group (moving-operand columns per matmul)
P = 128             # partitions
IC = I // P         # i chunks
HC = H // P         # h' chunks
FR = mybir.dt.float32r
F32 = mybir.dt.float32
BF = mybir.dt.bfloat16
NPBF = ml_dtypes.bfloat16

MASK_PAD = -30000.0


def _groups(l_pad):
    """Split l_pad into moving-operand column groups (<=512, mult of 128)."""
    offs = []
    off = 0
    while off < l_pad:
        n = min(LG, l_pad - off)
        offs.append((off, n))
        off += n
    return offs


def build_module(l_pad, b_pc: int = B_PC):
    """Build the per-core Bass module (same program on every core)."""
    assert l_pad % P == 0
    groups = _groups(l_pad)
    n_lt = l_pad // P   # l-tiles of 128 for pass 2 / e transpose

    nc = bacc.Bacc("TRN2", target_bir_lowering=False, debug=False,
                   enable_asserts=False, num_devices=N_CORES)

    hT_d = nc.dram_tensor("hT", (b_pc, I, l_pad), BF, kind="ExternalInput").ap()
    hN_d = nc.dram_tensor("hN", (b_pc, l_pad, I), BF, kind="ExternalInput").ap()
    w_d = nc.dram_tensor("w_hT", (I, H), BF, kind="ExternalInput").ap()
    vw_d = nc.dram_tensor("v_w", (H,), BF, kind="ExternalInput").ap()
    bias_d = nc.dram_tensor("bias", (H,), F32, kind="ExternalInput").ap()
    madd_d = nc.dram_tensor("m_add", (b_pc, l_pad), F32, kind="ExternalInput").ap()
    ones_d = nc.dram_tensor("ones97", (97, 1), FR, kind="ExternalInput").ap()
    reps_d = nc.dram_tensor("reps", (1, 1), mybir.dt.int32,
                            kind="ExternalInput").ap()
    r_d = nc.dram_tensor("r", (b_pc, I), F32, kind="ExternalOutput").ap()

    with tile.TileContext(nc) as tc, ExitStack() as ctx:
        const_p = ctx.enter_context(tc.tile_pool(name="const", bufs=1))
        hT_p = ctx.enter_context(tc.tile_pool(name="hT", bufs=16))
        tanh_p = ctx.enter_context(tc.tile_pool(name="tanh", bufs=20))
        hN_p = ctx.enter_context(tc.tile_pool(name="hN", bufs=6))
        small_p = ctx.enter_context(tc.tile_pool(name="small", bufs=2))
        pre_ps = ctx.enter_context(tc.tile_pool(name="preps", bufs=3, space="PSUM"))
        s_ps = ctx.enter_context(tc.tile_pool(name="sps", bufs=2, space="PSUM"))
        comb_ps = ctx.enter_context(tc.tile_pool(name="combps", bufs=2, space="PSUM"))
        r_ps = ctx.enter_context(tc.tile_pool(name="rps", bufs=1, space="PSUM"))
        dram_p = ctx.enter_context(tc.tile_pool(name="edram", bufs=2, space="DRAM"))

        # --- resident constants ---
        ones97 = const_p.tile([97, 1], FR, tag="ones97")
        nc.sync.dma_start(out=ones97[:], in_=ones_d)
        w_sb = const_p.tile([P, IC * H], BF, tag="W")  # [p, ic*H + h]
        nc.sync.dma_start(
            out=w_sb[:].rearrange("p (ic h) -> p ic h", ic=IC),
            in_=w_d.rearrange("(ic p) h -> p ic h", p=P),
        )
        vw_sb = const_p.tile([P, HC], BF, tag="vw")    # [q, hc]
        nc.sync.dma_start(out=vw_sb[:], in_=vw_d.rearrange("(hc q) -> q hc", q=P))
        bias_sb = const_p.tile([P, HC], F32, tag="bias")
        nc.sync.dma_start(out=bias_sb[:], in_=bias_d.rearrange("(hc q) -> q hc", q=P))

        def pass1(b):
            """Compute masked scores s for batch b into an sbuf tile (1, l_pad)."""
            m_sb = small_p.tile([1, l_pad], F32, tag="madd")
            nc.sync.dma_start(out=m_sb[:], in_=madd_d[b:b + 1, :])
            s_sb = small_p.tile([1, l_pad], F32, tag="s")

            def emit_spack(off, n, tanh_tiles):
                # 4-wide col-packed dot: chunks hc and hc+4 accumulate at
                # psum partition 32*(hc%4); the 4 col-groups run concurrently
                sgp = s_ps.tile([P, n], F32, tag="spack")
                # rows other than {0,32,64,96} are read (x0.0) by the combine
                # matmul; clear them so every read byte is written by this tile
                nc.vector.memset(sgp[:], 0.0)
                for hc in range(HC):
                    j = hc % 4
                    nc.tensor.matmul(
                        sgp[32 * j:32 * j + 1, :], vw_sb[:, hc:hc + 1],
                        tanh_tiles[hc][:],
                        start=(hc < 4), stop=(hc >= 4),
                        tile_position=(0, 32 * j))
                return sgp

            def emit_combine(off, n, sgp):
                part = small_p.tile([97, LG], FR, tag="spart")
                nc.vector.tensor_copy(part[:97, :n], sgp[0:97, :])
                sg = comb_ps.tile([1, n], F32, tag="scomb")
                nc.tensor.matmul(sg[:], ones97[:], part[:97, :n],
                                 start=True, stop=True)
                nc.vector.tensor_add(
                    s_sb[0:1, off:off + n], sg[:], m_sb[0:1, off:off + n])

            pend_s = []      # [(off, n, tanh_tiles)] awaiting packed-s emission
            pend_comb = []   # [(off, n, sgp)] awaiting combine emission
            for (off, n) in groups:
                hT_tiles = []
                for ic in range(IC):
                    t = hT_p.tile([P, n], BF, tag="hT")
                    nc.sync.dma_start(
                        out=t[:], in_=hT_d[b, ic * P:(ic + 1) * P, off:off + n])
                    hT_tiles.append(t)
                tanh_tiles = []
                for hc in range(HC):
                    pre = pre_ps.tile([P, n], F32, tag="pre")
                    for ic in range(IC):
                        nc.tensor.matmul(
                            pre[:],
                            w_sb[:, ic * H + hc * P: ic * H + (hc + 1) * P],
                            hT_tiles[ic][:],
                            start=(ic == 0), stop=(ic == IC - 1),
                        )
                    th = tanh_p.tile([P, n], BF, tag="tanh")
                    nc.scalar.activation(
                        th[:], pre[:], mybir.ActivationFunctionType.Tanh,
                        bias=bias_sb[:, hc:hc + 1], scale=1.0)
                    tanh_tiles.append(th)
                # one-group-deferred emission hides the ACT/DVE latencies
                # between the packed mms and the combine matmul
                if pend_comb:
                    emit_combine(*pend_comb.pop())
                if pend_s:
                    o2, n2, tt = pend_s.pop()
                    pend_comb.append((o2, n2, emit_spack(o2, n2, tt)))
                pend_s.append((off, n, tanh_tiles))
            while pend_s or pend_comb:
                if pend_comb:
                    emit_combine(*pend_comb.pop())
                if pend_s:
                    o2, n2, tt = pend_s.pop()
                    pend_comb.append((o2, n2, emit_spack(o2, n2, tt)))
            return s_sb

        def softmax(s_sb):
            """exp(s - max) -> e_col (l on partitions, bf16) + 1/sum(e)."""
            negm = small_p.tile([1, 1], F32, tag="negm")
            nc.vector.reduce_max(negm[:], s_sb[:], axis=mybir.AxisListType.X,
                                 negate=True)
            e_sb = small_p.tile([1, l_pad], BF, tag="e")
            d_sb = small_p.tile([1, 1], F32, tag="d")
            nc.scalar.activation(
                e_sb[:], s_sb[:], mybir.ActivationFunctionType.Exp,
                bias=negm[0:1, 0:1], scale=1.0, accum_out=d_sb[:])
            rd = small_p.tile([1, 1], F32, tag="rd")
            nc.vector.reciprocal(rd[:], d_sb[:])
            # transpose e (1, l_pad) -> (128, n_lt) via a DRAM round-trip
            e_dram = dram_p.tile([1, l_pad], BF, tag="edram")
            nc.sync.dma_start(out=e_dram[:], in_=e_sb[:])
            e_col = small_p.tile([P, n_lt], BF, tag="ecol")
            nc.sync.dma_start(
                out=e_col[:], in_=e_dram[:].rearrange("o (lt p) -> p (o lt)", p=P))
            return e_col, rd

        def pass2(b, e_col, rd):
            """r[b] = (1/d) * sum_l e_l h[b,l,:] via 4-wide col-packed PE."""
            r_sb = small_p.tile([1, I], F32, tag="rsb")
            nq = I // 256  # 4 col-groups of 256
            rpk = r_ps.tile([P, 256], F32, tag="rpack")
            for lt in range(n_lt):
                hn = hN_p.tile([P, I], BF, tag="hN")
                nc.sync.dma_start(out=hn[:], in_=hN_d[b, lt * P:(lt + 1) * P, :])
                for j in range(nq):
                    nc.tensor.matmul(
                        rpk[32 * j:32 * j + 1, :], e_col[:, lt:lt + 1],
                        hn[:, 256 * j:256 * (j + 1)],
                        start=(lt == 0), stop=(lt == n_lt - 1),
                        tile_position=(0, 32 * j))
            for j in range(nq):
                nc.vector.tensor_scalar_mul(
                    r_sb[0:1, 256 * j:256 * (j + 1)], rpk[32 * j:32 * j + 1, :],
                    rd[0:1, 0:1])
            nc.sync.dma_start(out=r_d[b:b + 1, :], in_=r_sb[:])

        # software pipeline: pass2(b) is emitted after pass1(b+1) so the PE
        # never waits on the softmax chain
        def body():
            pending = None
            for b in range(b_pc):
                s_sb = pass1(b)
                ecol_rd = softmax(s_sb)
                if pending is not None:
                    pass2(pending[0], *pending[1])
                pending = (b, ecol_rd)
            pass2(pending[0], *pending[1])

        reps_sb = const_p.tile([1, 1], mybir.dt.int32, tag="reps")
        nc.sync.dma_start(out=reps_sb[:], in_=reps_d)
        reps_val = nc.values_load(reps_sb[0:1, 0:1], min_val=1,
                                  max_val=1 << 20,
                                  skip_runtime_bounds_check=True)
        with tc.For_i(0, reps_val, 1):
            body()

    nc.compile()
    return nc


# ---------------------------------------------------------------------------
# Host-side runtime: shard, upload, execute via PJRT (axon), gather.
# ---------------------------------------------------------------------------

class _Runtime:
    def __init__(self, nc, n_cores=N_CORES):
        from concourse import bass2jax
        from jax.sharding import Mesh, PartitionSpec, NamedSharding
        from jax.experimental.shard_map import shard_map

        bass2jax.install_neuronx_cc_hook()
        self.nc = nc
        self.n_cores = n_cores

        partition_name = (nc.partition_id_tensor.name
                          if nc.partition_id_tensor else None)
        in_names, out_names, out_avals, zero_shapes = [], [], [], []
        for alloc in nc.m.functions[0].allocations:
            if not isinstance(alloc, mybir.MemoryLocationSet):
                continue
            name = alloc.memorylocations[0].name
            if alloc.kind == "ExternalInput":
                if name != partition_name:
                    in_names.append(name)
            elif alloc.kind == "ExternalOutput":
                shape = tuple(alloc.tensor_shape)
                dtype = mybir.dt.np(alloc.dtype)
                out_names.append(name)
                out_avals.append(jax.core.ShapedArray(shape, dtype))
                zero_shapes.append((shape, dtype))
        self.in_names = list(in_names)
        self.out_names = out_names
        self.out_avals = out_avals
        self.zero_shapes = zero_shapes
        n_params = len(in_names)
        n_outs = len(out_names)
        all_names = in_names + out_names
        if partition_name is not None:
            all_names = all_names + [partition_name]

        from concourse.bass2jax import _bass_exec_p, partition_id_tensor

        def _body(*args):
            operands = list(args)
            if partition_name is not None:
                operands.append(partition_id_tensor())
            outs = _bass_exec_p.bind(
                *operands,
                out_avals=tuple(out_avals),
                in_names=tuple(all_names),
                out_names=tuple(out_names),
                lowering_input_output_aliases=(),
                sim_require_finite=False,
                sim_require_nnan=False,
                nc=nc,
            )
            return tuple(outs)

        devices = jax.devices()[:n_cores]
        self.mesh = Mesh(np.asarray(devices), ("core",))
        pspec = PartitionSpec("core")
        self.sharding = NamedSharding(self.mesh, pspec)
        donate = tuple(range(n_params, n_params + n_outs))
        self.fn = jax.jit(
            shard_map(_body, mesh=self.mesh,
                      in_specs=(pspec,) * (n_params + n_outs),
                      out_specs=(pspec,) * n_outs,
                      check_rep=False),
            donate_argnums=donate, keep_unused=True)

    def put_inputs(self, in_maps):
        concat = [
            np.concatenate([np.asarray(m[name]) for m in in_maps], axis=0)
            for name in self.in_names
        ]
        return [jax.device_put(a, self.sharding) for a in concat]

    def run(self, dev_inputs):
        zeros = [
            jax.device_put(np.zeros((self.n_cores * s[0], *s[1:]), dt), self.sharding)
            for s, dt in self.zero_shapes
        ]
        outs = self.fn(*dev_inputs, *zeros)
        jax.block_until_ready(outs)
        return outs

    def gather(self, outs):
        res = []
        for c in range(self.n_cores):
            d = {}
            for i, name in enumerate(self.out_names):
                d[name] = np.asarray(outs[i]).reshape(
                    self.n_cores, *self.out_avals[i].shape)[c]
            res.append(d)
        return res


_CACHE = {}


def _get_runtime(l_pad=None):
    if l_pad is None:
        # test-harness convenience: return the most recently built runtime
        assert _CACHE, "call kernel()/prep first"
        return next(iter(_CACHE.values()))
    if l_pad not in _CACHE:
        nc = build_module(l_pad)
        _CACHE[l_pad] = _Runtime(nc)
    return _CACHE[l_pad]


def _selections(mask):
    """Per-batch unmasked index lists + uniform padded length."""
    sels = []
    for b in range(mask.shape[0]):
        sel = np.nonzero(mask[b])[0]
        if sel.size == 0:
            # fully masked: softmax(s - 1e4) == softmax(s); keep all rows
            sel = np.arange(mask.shape[1])
        sels.append(sel)
    l_max = max(s.size for s in sels)
    l_pad = max(P, int(np.ceil(l_max / P)) * P)
    return sels, l_pad


def prep_in_maps(h, mask, ln_w, ln_b, v_w, vq, reps=1):
    """Host-side preprocessing + sharding into per-core input maps."""
    h = np.asarray(h, dtype=np.float32)
    mask = np.asarray(mask)
    ln_w = np.asarray(ln_w, dtype=np.float32)
    ln_b = np.asarray(ln_b, dtype=np.float32)
    v_w = np.asarray(v_w, dtype=np.float32)
    vq = np.asarray(vq, dtype=np.float32)

    sels, l_pad = _selections(mask)

    w_hT = np.ascontiguousarray(ln_w[:, :I].T).astype(NPBF)   # (I, H)
    bias = (ln_b + ln_w[:, I:] @ vq).astype(np.float32)       # (H,)
    vw_bf = v_w.astype(NPBF)
    ones97 = np.zeros((97, 1), np.float32)
    ones97[::32] = 1.0

    hN = np.zeros((B_FULL, l_pad, I), NPBF)
    m_add = np.full((B_FULL, l_pad), MASK_PAD, np.float32)
    for b in range(B_FULL):
        n = sels[b].size
        hN[b, :n] = h[b][sels[b]].astype(NPBF)
        m_add[b, :n] = 0.0

    in_maps = []
    for c in range(N_CORES):
        lo, hi = c * B_PC, (c + 1) * B_PC
        hb = hN[lo:hi]
        in_maps.append({
            "hT": np.ascontiguousarray(hb.transpose(0, 2, 1)),
            "hN": hb,
            "w_hT": w_hT,
            "v_w": vw_bf,
            "bias": bias,
            "m_add": m_add[lo:hi],
            "ones97": ones97,
            "reps": np.full((1, 1), reps, np.int32),
        })
    return in_maps, l_pad


def kernel(h, mask, ln_w, ln_b, v_w, vq):
    in_maps, l_pad = prep_in_maps(h, mask, ln_w, ln_b, v_w, vq)
    rt = _get_runtime(l_pad)
    last_err = None
    for attempt in range(3):
        try:
            dev = rt.put_inputs(in_maps)
            outs = rt.run(dev)
            res = rt.gather(outs)
            return np.concatenate([res[c]["r"] for c in range(N_CORES)], axis=0)
        except Exception as e:  # transient device-unrecoverable flakes
            last_err = e
            _CACHE.pop(l_pad, None)
            jax.clear_caches()
            rt = _get_runtime(l_pad)
    raise last_err
